# revision 12
# baseline (speedup 1.0000x reference)
"""Trainium2 Bass kernel for the ChaosModulator recurrence.

Math (per (b,c) sequence, t = 0..4095):
    sigma_t = 3.5*z*(1-z) + 0.5*x_t
    z'      = 0.5*z + 0.25*(1 + tanh(sigma_t))        (clip is a no-op: z' in (0,1))
    u_t     = 0.5*x_t + 0.5*(2*z' - 1)

Reformulated in sigma-space (sigma = (2z-1+h)/4):
    t_k   = 0.875 + 0.5*x_k - 3.5*sigma_{k-1}^2
    sig_k = 0.5*sigma_{k-1} + 0.25*tanh(t_k)
    u_k   = sigma_k + 0.5*x_k

tanh is replaced by a density-weighted degree-5 odd polynomial with output
clamp (h* = clamp(P*(tau), +-0.25), tau = lam*t), validated end-to-end in
fp32: rel err 2.3e-3 vs fp64 sequential reference (gate 2e-2).  This fits
the whole step into TWO fused DVE ops (8-stage datapath, 3 constants each,
1 elem/cycle) with no Activation-engine dependency, so the serial chain
never leaves the DVE engine and pays no cross-engine semaphore latency.

The map contracts ~0.5/step, so each 128-step time block is computed
independently after an 8-step zero-state warmup (warmup error < 3e-7).
That turns the serial t-loop into 32 independent chains per sequence ->
wide [128 x 128] per-step ops.

States go to a 16-slot ring; u = 0.5*x + sigma is computed in bulk every
8 steps on the otherwise-idle Pool (GPSIMD) engine, written in-place over
the consumed x slots, then DMAd out on the Activation HWDGE queue (input
loads use the SP queue).  All DMA runs are >= 512B so no descriptor
latency penalty applies.

Sharding: batch dim b (32) split 4-per-core across 8 cores; per core
2048 sequences x 4096 steps.
"""

import numpy as np

import concourse.bacc as bacc
import concourse.dve_ops as dve_ops
import concourse.mybir as mybir
from concourse.bass_utils import run_bass_kernel_spmd
from concourse.dve_spec import (
    C0, C1, C2, One, Spec, Src0, Src1, Zero, _has_src1, lower, maxx, minn, sq,
)
from concourse.dve_uop import DveOpSpec
from concourse.tile import TileContext

F32 = mybir.dt.float32
P = 128             # SBUF partitions
G = 16              # sequence groups per core (2048 = G*P)
T = 4096
B = 64              # output steps per block
W = 8               # warmup steps per block
L = B + W           # chain length (72)
NBLK = T // B       # 64 blocks
BLK_PER_BATCH = 16
NBATCH = NBLK // BLK_PER_BATCH   # 4
NCOLS = BLK_PER_BATCH * G        # 256 chain-columns per batch
NSEQ = P * G        # 2048 sequences per core
NCORES = 8
XBUFS = 2
RING = 16           # state ring depth (multiple of 8, > W)

# tanh(t) ~ A*t + B3*t^3 + C5*t^5 (density-weighted LSQ fit constrained to
# P(2/A) = 1, monotone).  In tau = LAM*t units (LAM = A/2) the crossing is
# at tau = 1, so clamping tau to +-1 in op1 equals clamping P* to +-0.25.
# P*(tau) = 0.5*tau + tau*y*(PC1 + PC2*y), y = tau^2.
FIT_A = 0.97319589
FIT_B = -0.23358437
FIT_C = 2.80270987e-02
LAM = FIT_A / 2.0
PC1 = FIT_B / (4.0 * LAM**3)
PC2 = FIT_C / (4.0 * LAM**5)
# op1 constants: tau = clamp(T_C2 + T_C1*xh - T_C0*sigma^2, +-1) where
# xh = 0.5*x (x is pre-halved in SBUF by the Pool engine so the bulk
# u = xh + sigma becomes a plain Pool tensor_tensor add).
T_C0 = 3.5 * LAM
T_C1 = LAM
T_C2 = 0.875 * LAM

_MULT = mybir.AluOpType.mult
_ADD = mybir.AluOpType.add


def _register_custom_ops():
    """Register the fused DVE ops (idempotent)."""
    if "CHAOS_T" in dve_ops._SUB_OPCODE_FOR_NAME:
        by = {op.name: op for op in dve_ops.OPS}
        return by["CHAOS_T"], by["CHAOS_P"]

    # op1: tau = clamp(C2 + C1*x - C0*sigma^2, +-1)   (in0=sigma, in1=x)
    spec_t = Spec(
        body=minn(maxx((C1 * Src1 - C0 * sq(Src0)) + C2, Zero - One), One),
        reference=lambda in0, in1, s0, s1, imm2: np.minimum(
            np.maximum((s1 * in1 - s0 * in0 * in0) + imm2, -1.0), 1.0
        ),
    )

    # op2: sigma' = C0*sigma + C0*tau + tau*y*(C1 + C2*y),  y = tau^2.
    #      P* is monotone with P*(+-1) = +-0.25, so the op1 tau-clamp
    #      already bounds the h contribution to +-0.25.  (in0=tau, in1=sigma)
    y = sq(Src0)
    pstar = Src0 * (C0 + y * (C1 + C2 * y))
    spec_p = Spec(
        body=C0 * Src1 + pstar,
        reference=lambda in0, in1, s0, s1, imm2: s0 * in1
        + s0 * in0
        + (in0 * in0 * in0) * (s1 + imm2 * in0 * in0),
    )

    ops = []
    for name, spec in (("CHAOS_T", spec_t), ("CHAOS_P", spec_p)):
        op = dve_ops.DveOp(name, spec, subdim=False, uops_sha={})
        dve_ops.OPS.append(op)
        dve_ops.CUSTOM_DVE_SPECS[name] = spec
        dve_ops._SUB_OPCODE_FOR_NAME[name] = (
            dve_ops._CUSTOM_DVE_ROW_BASE + len(dve_ops.OPS) - 1
        )
        for ver in ("v3", "v4"):
            try:
                s = DveOpSpec(
                    name=name,
                    opcode=dve_ops.get_dve_sub_opcode(name),
                    uops=lower(spec, ver=ver),
                    rd1_en=_has_src1(spec),
                )
                op.uops_sha[ver] = s.sha(ver)
            except Exception:
                pass
        ops.append(op)
    return ops


def _build_nc(reps: int = 1):
    CHAOS_T, CHAOS_P = _register_custom_ops()
    nc = bacc.Bacc("TRN2", target_bir_lowering=False, debug=False)

    x = nc.dram_tensor("x", [NSEQ, T], F32, kind="ExternalInput")
    z0 = nc.dram_tensor("z0", [NSEQ], F32, kind="ExternalInput")
    u = nc.dram_tensor("u", [NSEQ, T], F32, kind="ExternalOutput")

    xr = x[:, :].rearrange("(g p) t -> p g t", p=P)    # [128, 16, 4096]
    ur = u[:, :].rearrange("(g p) t -> p g t", p=P)
    z0r = z0[:].rearrange("(g p) -> p g", p=P)         # [128, 16]

    with TileContext(nc) as tc:
        with (
            tc.tile_pool(name="xp", bufs=XBUFS) as xp,
            tc.tile_pool(name="rp", bufs=XBUFS) as rp,
            tc.tile_pool(name="sp", bufs=XBUFS) as sp,
            tc.tile_pool(name="cp", bufs=1) as cp,
        ):
            z0_t = cp.tile([P, G], F32)
            nc.sync.dma_start(out=z0_t[:, :], in_=z0r)
            # sigma_init = z0 - 0.5
            sig_init = cp.tile([P, G], F32)
            nc.vector.tensor_scalar(
                out=sig_init[:, :], in0=z0_t[:, :],
                scalar1=1.0, scalar2=-0.5, op0=_MULT, op1=_ADD,
            )

            for rep in range(reps):
             for bt in range(NBATCH):
                # X tile: [c][k] layout, c = nl*G + g, k = chain step.
                # Slots hold raw x; u is written in-place over x as steps
                # retire (u_j -> slot j for j >= W).
                Xt = xp.tile([P, NCOLS * L], F32, name=f"X{bt}", tag="X")
                Xv = Xt.rearrange("p (c k) -> p c k", k=L)

                for nl in range(BLK_PER_BATCH):
                    n = bt * BLK_PER_BATCH + nl
                    cs = nl * G
                    if n == 0:
                        # block 0 starts at t=-W: pad warmup with zeros
                        nc.vector.memset(Xv[:, cs:cs + G, 0:W], 0.0)
                        nc.sync.dma_start(
                            out=Xv[:, cs:cs + G, W:L], in_=xr[:, :, 0:B]
                        )
                    else:
                        t0 = n * B - W
                        nc.sync.dma_start(
                            out=Xv[:, cs:cs + G, :], in_=xr[:, :, t0:t0 + L]
                        )
                    # pre-halve this block's x on Pool (overlaps later loads)
                    nc.gpsimd.tensor_scalar(
                        out=Xv[:, cs:cs + G, :], in0=Xv[:, cs:cs + G, :],
                        scalar1=0.5, scalar2=0.0, op0=_MULT, op1=_ADD,
                    )

                # state ring: sigma_k lives in slot k % RING
                ring_t = rp.tile([P, NCOLS * RING], F32, name=f"R{bt}", tag="R")
                ring = ring_t.rearrange("p (c r) -> p c r", r=RING)
                # sigma_{-1} = 0
                nc.vector.memset(ring[:, :, RING - 1], 0.0)

                tau_t = sp.tile([P, NCOLS], F32, name=f"t{bt}", tag="tau")

                for k in range(L):
                    rk = k % RING
                    rkm1 = (k - 1) % RING
                    # tau = T_C2 + T_C1*x_k - T_C0*sigma_{k-1}^2
                    nc.vector._custom_dve(
                        CHAOS_T, out=tau_t[:, :], in0=ring[:, :, rkm1],
                        in1=Xv[:, :, k], s0=T_C0, s1=T_C1, imm2=T_C2,
                    )
                    # sigma_k = 0.5*sigma_{k-1} + clamp(P*(tau), +-0.25)
                    nc.vector._custom_dve(
                        CHAOS_P, out=ring[:, :, rk], in0=tau_t[:, :],
                        in1=ring[:, :, rkm1], s0=0.5, s1=PC1, imm2=PC2,
                    )
                    if bt == 0 and k == W - 1:
                        # block 0: replace warmup state with the true z0 state
                        nc.vector.tensor_copy(
                            out=ring[:, 0:G, rk], in_=sig_init[:, :]
                        )
                    # bulk u for j in [k-7, k]: u_j = xh_j + sigma_j,
                    # in-place over the pre-halved x slots (last read by
                    # CHAOS_T at step j; ring slots live RING=16 steps).
                    if k >= 2 * W - 1 and (k + 1) % W == 0:
                        a = k + 1 - W
                        ra = a % RING
                        nc.gpsimd.tensor_tensor(
                            out=Xv[:, :, a:a + W], in0=Xv[:, :, a:a + W],
                            in1=ring[:, :, ra:ra + W], op=_ADD,
                        )

                for nl in range(BLK_PER_BATCH):
                    n = bt * BLK_PER_BATCH + nl
                    cs = nl * G
                    nc.scalar.dma_start(
                        out=ur[:, :, n * B:(n + 1) * B],
                        in_=Xv[:, cs:cs + G, W:L],
                    )

    nc.compile()
    return nc


_NC = None


def _get_nc():
    global _NC
    if _NC is None:
        _NC = _build_nc()
    return _NC


def kernel(x: np.ndarray, z0: np.ndarray) -> np.ndarray:
    x = np.ascontiguousarray(x, dtype=np.float32)      # (32, 512, 4096)
    z0 = np.ascontiguousarray(z0, dtype=np.float32)    # (32, 512)
    nc = _get_nc()

    in_maps = []
    for i in range(NCORES):
        xs = np.ascontiguousarray(x[4 * i:4 * (i + 1)].reshape(NSEQ, T))
        zs = np.ascontiguousarray(z0[4 * i:4 * (i + 1)].reshape(NSEQ))
        in_maps.append({"x": xs, "z0": zs})

    res = run_bass_kernel_spmd(nc, in_maps, core_ids=list(range(NCORES)))
    out = np.empty((32, 512, T), np.float32)
    for i in range(NCORES):
        out[4 * i:4 * (i + 1)] = res.results[i]["u"].reshape(4, 512, T)
    return out
